# revision 2
# baseline (speedup 1.0000x reference)
"""DIEN attention-GRU kernel v2 for 8 trn2 NeuronCores.

Data-parallel over batch (1024 -> 128/core). Feature-major layout
([H, batch]) so gate biases ride per-partition scalar slots and no
per-step transpose is needed. The scan runs as TWO interleaved
half-batch streams (64 each) to hide the serial dependency chain.
All matmul operands are bf16 (PE 1 cyc/row); PSUM accumulation f32.

Per stream-step PSUM bank [H, 512 f32]: regions
  u: 0:64    pre_u = bu + Wxu.hist + Whu.h      (bias mm, xfill mm, h mm)
  r: 64:128  pre_r = br + Wxr.hist + Whr.h
  m: 128:192 Whg.h
  Z: 192:256 gin(DVE) + Wxg.hist(mm accumulate); tanh reads Z with bgx bias
Math per step (exact GRU):
  sru = sigmoid([pre_u | pre_r])
  gin = (m + bg) * sig_r          (DVE STT, writes Z)
  Z  += Wxg.hist_t                (PE accumulate onto DVE data)
  g   = tanh(Z + bgx)             (Act bias arg)
  v   = sig_u * a_bc              (Pool; a_bc = partition-broadcast att)
  h'  = h + v * (g - h)           (DVE TTs)
"""

import sys

sys.path.insert(0, "/opt/trn_rl_repo")

import numpy as np
import ml_dtypes

import concourse.bacc as bacc
import concourse.mybir as mybir
from concourse.tile import TileContext
from concourse.tile_rust import add_dep_helper
from concourse.bass_utils import run_bass_kernel_spmd

B, T, IN, H = 1024, 200, 128, 128
NCORES = 8
BS = B // NCORES          # 128 batch per core
SW = BS // 2              # 64 per stream
G = 8                     # phase-1 time group
NG = T // G

F32 = mybir.dt.float32
BF16 = mybir.dt.bfloat16
AF = mybir.ActivationFunctionType
ALU = mybir.AluOpType
bf16 = ml_dtypes.bfloat16

# psum bank f32 column offsets
U0, R0, M0, Q0 = 0, 64, 128, 192


def build_nc(t_steps=T, num_devices=NCORES, debug=False):
    nc = bacc.Bacc("TRN2", target_bir_lowering=False, debug=False,
                   num_devices=num_devices)
    ng = t_steps // G
    assert t_steps % G == 0

    histT = nc.dram_tensor("histT", [H, t_steps, BS], BF16, kind="ExternalInput")
    tgtT = nc.dram_tensor("tgtT", [IN, t_steps, BS], BF16, kind="ExternalInput")
    wWT = nc.dram_tensor("wWT", [IN, H], BF16, kind="ExternalInput")
    wb_col = nc.dram_tensor("wb_col", [H, 1], F32, kind="ExternalInput")
    WhuT = nc.dram_tensor("WhuT", [H, H], BF16, kind="ExternalInput")
    WhrT = nc.dram_tensor("WhrT", [H, H], BF16, kind="ExternalInput")
    WhgT = nc.dram_tensor("WhgT", [H, H], BF16, kind="ExternalInput")
    WxuT = nc.dram_tensor("WxuT", [H, H], BF16, kind="ExternalInput")
    WxrT = nc.dram_tensor("WxrT", [H, H], BF16, kind="ExternalInput")
    WxgT = nc.dram_tensor("WxgT", [H, H], BF16, kind="ExternalInput")
    bias2 = nc.dram_tensor("bias2", [2, H], BF16, kind="ExternalInput")
    mask2 = nc.dram_tensor("mask2", [2, 4 * SW], BF16, kind="ExternalInput")
    bg_col = nc.dram_tensor("bg_col", [H, 1], F32, kind="ExternalInput")
    bgx_col = nc.dram_tensor("bgx_col", [H, 1], F32, kind="ExternalInput")
    ln2wh = nc.dram_tensor("ln2wh", [H, H], BF16, kind="ExternalInput")
    ln2wt = nc.dram_tensor("ln2wt", [IN, H], BF16, kind="ExternalInput")
    ln2b_row = nc.dram_tensor("ln2b_row", [1, H], BF16, kind="ExternalInput")
    ones_row = nc.dram_tensor("ones_row", [1, BS], BF16, kind="ExternalInput")
    onesH2 = nc.dram_tensor("onesH2", [H, 2], BF16, kind="ExternalInput")
    attd = nc.dram_tensor("attd", [BS, t_steps], BF16, kind="Internal")
    out_d = nc.dram_tensor("out", [BS, H], F32, kind="ExternalOutput")
    if debug:
        dbg_att = nc.dram_tensor("dbg_att", [BS, t_steps], F32,
                                 kind="ExternalOutput")
        dbg_abc = nc.dram_tensor("dbg_abc", [4, H, SW], F32,
                                 kind="ExternalOutput")
        dbg_h = nc.dram_tensor("dbg_h", [4, H, SW], F32,
                               kind="ExternalOutput")
        dbg_sru = nc.dram_tensor("dbg_sru", [4, H, 2 * SW], F32,
                                 kind="ExternalOutput")
        dbg_g = nc.dram_tensor("dbg_g", [4, H, SW], F32,
                               kind="ExternalOutput")

    with TileContext(nc) as tc:
        with (
            tc.tile_pool(name="const", bufs=1) as constp,
            tc.tile_pool(name="hist", bufs=1) as histp,
            tc.tile_pool(name="tgt", bufs=3) as tgtp,
            tc.tile_pool(name="prod", bufs=2) as prodp,
            tc.tile_pool(name="attp", bufs=1) as attp,
            tc.tile_pool(name="scan", bufs=3) as scanp,
            tc.tile_pool(name="state", bufs=2) as statep,
            tc.tile_pool(name="awps", bufs=2, space="PSUM") as awps,
            tc.tile_pool(name="lgps", bufs=1, space="PSUM") as lgps,
            tc.tile_pool(name="bkA", bufs=2, space="PSUM") as bkAp,
            tc.tile_pool(name="bkB", bufs=2, space="PSUM") as bkBp,
            tc.tile_pool(name="trps", bufs=1, space="PSUM") as trps,
        ):
            def cload(dram, shape, dt):
                t = constp.tile(shape, dt, tag=dram.name)
                nc.sync.dma_start(t[:], dram[:, :])
                return t

            wWT_s = cload(wWT, [IN, H], BF16)
            wb_s = cload(wb_col, [H, 1], F32)
            WhuT_s = cload(WhuT, [H, H], BF16)
            WhrT_s = cload(WhrT, [H, H], BF16)
            WhgT_s = cload(WhgT, [H, H], BF16)
            WxuT_s = cload(WxuT, [H, H], BF16)
            WxrT_s = cload(WxrT, [H, H], BF16)
            WxgT_s = cload(WxgT, [H, H], BF16)
            bias2_s = cload(bias2, [2, H], BF16)
            mask2_s = cload(mask2, [2, 4 * SW], BF16)
            bg_s = cload(bg_col, [H, 1], F32)
            bgx_s = cload(bgx_col, [H, 1], F32)
            ln2wh_s = cload(ln2wh, [H, H], BF16)
            ln2wt_s = cload(ln2wt, [IN, H], BF16)
            ln2b_s = cload(ln2b_row, [1, H], BF16)
            ones_s = cload(ones_row, [1, BS], BF16)
            onesH2_s = cload(onesH2, [H, 2], BF16)

            onesT = constp.tile([H, SW], BF16, tag="onesT")
            nc.vector.memset(onesT[:], 1.0)
            hist_all = histp.tile([H, t_steps, BS], BF16, tag="hist")
            logits_ps = lgps.tile([BS, t_steps, 2], F32, tag="logits")

            # ---- phase 1: stream inputs, attention logits ----
            aw_tiles = {}

            def p1_load(g):
                t0 = g * G
                nc.sync.dma_start(hist_all[:, t0:t0 + G, :],
                                  histT[:, t0:t0 + G, :])
                tg = tgtp.tile([IN, G, BS], BF16, tag="tgt")
                nc.sync.dma_start(tg[:], tgtT[:, t0:t0 + G, :])
                aw = []
                for hlf in range(2):
                    awt = awps.tile([H, (G // 2) * BS], F32, tag="aw")
                    nc.tensor.matmul(
                        awt[:], wWT_s[:],
                        tg[:, hlf * (G // 2):(hlf + 1) * (G // 2), :]
                        .rearrange("i t b -> i (t b)"),
                        start=True, stop=True)
                    aw.append(awt)
                aw_tiles[g] = aw

            def p1_reduce(g):
                t0 = g * G
                aw = aw_tiles.pop(g)
                for hlf in range(2):
                    prod = prodp.tile([H, (G // 2) * BS], BF16, tag="prod")
                    nc.vector.scalar_tensor_tensor(
                        prod[:], aw[hlf][:], wb_s[:],
                        hist_all[:, t0 + hlf * (G // 2):t0 + (hlf + 1) * (G // 2), :]
                        .rearrange("h t b -> h (t b)"),
                        ALU.add, ALU.mult)
                    for j in range(G // 2):
                        t = t0 + hlf * (G // 2) + j
                        nc.tensor.matmul(
                            logits_ps[:, t, :],
                            prod[:, j * BS:(j + 1) * BS],
                            onesH2_s[:],
                            start=True, stop=True)

            P1LA = 2
            for g in range(-P1LA, ng):
                if g + P1LA < ng:
                    p1_load(g + P1LA)
                if g >= 0:
                    p1_reduce(g)

            # ---- softmax over time (batch-major [BS, T]) ----
            mx = attp.tile([BS, 1], F32, tag="mx")
            nc.vector.tensor_reduce(mx[:], logits_ps[:, :, 0],
                                    mybir.AxisListType.X, ALU.max)
            negmx = attp.tile([BS, 1], F32, tag="negmx")
            nc.vector.tensor_scalar_mul(negmx[:], mx[:], -1.0)
            exps = attp.tile([BS, t_steps], F32, tag="exps")
            nc.scalar.activation(exps[:], logits_ps[:, :, 0], AF.Exp,
                                 bias=negmx[:])
            ssum = attp.tile([BS, 1], F32, tag="ssum")
            nc.vector.tensor_reduce(ssum[:], exps[:], mybir.AxisListType.X,
                                    ALU.add)
            rsum = attp.tile([BS, 1], F32, tag="rsum")
            nc.vector.reciprocal(rsum[:], ssum[:])
            att = attp.tile([BS, t_steps], F32, tag="att")
            nc.vector.tensor_scalar_mul(att[:], exps[:], rsum[:])

            # att rows via DRAM round-trip: att [BS, T] -> attd -> attRow
            # with stream A rows at partition 0, stream B at partition 64.
            if debug:
                nc.sync.dma_start(dbg_att[:, :], att[:])
            att_bf = attp.tile([BS, t_steps], BF16, tag="att_bf")
            nc.vector.tensor_copy(att_bf[:], att[:])
            nc.sync.dma_start(attd[:, :], att_bf[:])
            TCH = 50  # att broadcast chunk (steps)
            abcA = attp.tile([128, SW, t_steps], BF16, tag="abcA")
            abcB = attp.tile([128, SW, t_steps], BF16, tag="abcB")
            abcS = [abcA, abcB]

            def att_chunk(c):
                t0 = c * TCH
                for s in range(2):
                    nc.sync.dma_start(
                        abcS[s][:, :, t0:t0 + TCH],
                        attd[s * SW:(s + 1) * SW, t0:t0 + TCH]
                        .rearrange("(o b) t -> o b t", o=1)
                        .broadcast_to([128, SW, TCH]))

            # ---- phase 2: the scan, 2 interleaved streams ----
            bk_pools = (bkAp, bkBp)

            h_t = [None, None]
            for s in range(2):
                h0 = statep.tile([H, SW], BF16, tag=f"h{s}")
                nc.vector.memset(h0[:], 0.0)
                h_t[s] = h0

            banks = {}

            def sl(s):
                return slice(s * SW, (s + 1) * SW)

            def prefetch(s, t):
                bk = bk_pools[s].tile([H, 512], F32, tag=f"bk{s}")
                hs = hist_all[:, t, sl(s)]
                # single start=True mm claims the whole accumulated span
                # [u|r|m|q]: biases into u/r via masks, zeros into m/q.
                nc.tensor.matmul(bk[:, 0:4 * SW], bias2_s[:], mask2_s[:],
                                 start=True, stop=False)
                nc.tensor.matmul(bk[:, U0:U0 + SW], WxuT_s[:], hs,
                                 start=False, stop=False)
                nc.tensor.matmul(bk[:, R0:R0 + SW], WxrT_s[:], hs,
                                 start=False, stop=False)
                nc.tensor.matmul(bk[:, Q0:Q0 + SW], WxgT_s[:], hs,
                                 start=False, stop=False)
                banks[(s, t)] = bk

            def onchain(s, t):
                bk = banks.pop((s, t))
                h = h_t[s]
                abc_t = abcS[s][:, :, t]
                nc.tensor.matmul(bk[:, U0:U0 + SW], WhuT_s[:], h[:],
                                 start=False, stop=False)
                nc.tensor.matmul(bk[:, R0:R0 + SW], WhrT_s[:], h[:],
                                 start=False, stop=False)
                nc.tensor.matmul(bk[:, M0:M0 + SW], WhgT_s[:], h[:],
                                 start=False, stop=True)
                sru = scanp.tile([H, 2 * SW], BF16, tag=f"sru{s}")
                nc.scalar.activation(sru[:], bk[:, U0:U0 + 2 * SW], AF.Sigmoid)
                su = sru[:, 0:SW]
                gin = scanp.tile([H, SW], BF16, tag=f"gin{s}")
                nc.vector.scalar_tensor_tensor(
                    gin[:], bk[:, M0:M0 + SW], bg_s[:],
                    sru[:, SW:2 * SW], ALU.add, ALU.mult)
                gpre = scanp.tile([H, SW], BF16, tag=f"gpre{s}")
                nc.vector.tensor_tensor(gpre[:], bk[:, Q0:Q0 + SW], gin[:],
                                        ALU.add)
                g_ = scanp.tile([H, SW], BF16, tag=f"g{s}")
                nc.scalar.activation(g_[:], gpre[:], AF.Tanh, bias=bgx_s[:])
                v = scanp.tile([H, SW], BF16, tag=f"v{s}")
                nc.gpsimd.tensor_tensor(v[:], su, abc_t, ALU.mult)
                if debug and s == 0 and t < 4:
                    dt_ = scanp.tile([H, SW], F32, tag="dbga")
                    nc.vector.tensor_copy(dt_[:], abcS[s][:, :, t])
                    nc.sync.dma_start(dbg_abc[t, :, :], dt_[:])
                    ds_ = scanp.tile([H, 2 * SW], F32, tag="dbgs")
                    nc.vector.tensor_copy(ds_[:], sru[:])
                    nc.sync.dma_start(dbg_sru[t, :, :], ds_[:])
                    dg_ = scanp.tile([H, SW], F32, tag="dbgg")
                    nc.vector.tensor_copy(dg_[:], g_[:])
                    nc.sync.dma_start(dbg_g[t, :, :], dg_[:])
                s_ = scanp.tile([H, SW], BF16, tag=f"s{s}")
                nc.vector.tensor_tensor(s_[:], g_[:], h[:], ALU.subtract)
                tt = scanp.tile([H, SW], BF16, tag=f"tt{s}")
                nc.vector.tensor_tensor(tt[:], v[:], s_[:], ALU.mult)
                h2 = statep.tile([H, SW], BF16, tag=f"h{s}")
                nc.vector.tensor_tensor(h2[:], h[:], tt[:], ALU.add)
                if debug and s == 0 and t < 4:
                    dh_ = scanp.tile([H, SW], F32, tag="dbgh")
                    nc.vector.tensor_copy(dh_[:], h2[:])
                    nc.sync.dma_start(dbg_h[t, :, :], dh_[:])
                h_t[s] = h2

            # software-pipelined interleave: streams offset by half a step
            att_chunk(0)
            LA = 1  # prefetch lookahead in steps
            for hs_i in range(2 * (t_steps + LA)):
                s = hs_i % 2
                t = hs_i // 2
                if s == 0 and t + TCH // 5 in (TCH, 2 * TCH, 3 * TCH):
                    att_chunk((t + TCH // 5) // TCH)
                if t < t_steps:
                    prefetch(s, t)
                if t - LA >= 0:
                    onchain(s, t - LA)

            # ---- phase 3: out = [h, tgt0] @ ln2^T + b ----
            t0sb = scanp.tile([IN, BS], BF16, tag="t0sb")
            nc.sync.dma_start(t0sb[:], tgtT[:, 0, :])
            for s in range(2):
                opst = trps.tile([128, BS], F32, tag="misc")
                ops = opst[0:SW, :]
                nc.tensor.matmul(ops, h_t[s][:], ln2wh_s[:],
                                 start=True, stop=False)
                nc.tensor.matmul(ops, t0sb[:, sl(s)], ln2wt_s[:],
                                 start=False, stop=False)
                nc.tensor.matmul(ops, ones_s[:, 0:SW], ln2b_s[:],
                                 start=False, stop=True)
                out_s = scanp.tile([SW, H], F32, tag="out_s")
                nc.vector.tensor_copy(out_s[:], ops)
                nc.sync.dma_start(out_d[sl(s), :], out_s[:])

    nc.compile()
    return nc


def _mask2():
    m = np.zeros((2, 4 * SW), np.float32)
    m[0, 0:SW] = 1.0
    m[1, SW:2 * SW] = 1.0
    return m.astype(bf16)


def make_weight_feeds(inputs, t_steps=T):
    f32 = np.float32
    a = np.asarray
    feeds = {
        "wWT": np.ascontiguousarray(a(inputs["W_w"]).T).astype(bf16),
        "wb_col": a(inputs["W_b"], dtype=f32).reshape(H, 1).copy(),
        "WhuT": np.ascontiguousarray(a(inputs["hu_w"]).T).astype(bf16),
        "WhrT": np.ascontiguousarray(a(inputs["hr_w"]).T).astype(bf16),
        "WhgT": np.ascontiguousarray(a(inputs["hg_w"]).T).astype(bf16),
        "WxuT": np.ascontiguousarray(a(inputs["xu_w"]).T).astype(bf16),
        "WxrT": np.ascontiguousarray(a(inputs["xr_w"]).T).astype(bf16),
        "WxgT": np.ascontiguousarray(a(inputs["xg_w"]).T).astype(bf16),
        "bias2": np.stack([a(inputs["xu_b"]) + a(inputs["hu_b"]),
                           a(inputs["xr_b"]) + a(inputs["hr_b"])]).astype(bf16),
        "mask2": _mask2(),
        "bg_col": a(inputs["hg_b"], dtype=f32).reshape(H, 1).copy(),
        "bgx_col": a(inputs["xg_b"], dtype=f32).reshape(H, 1).copy(),
        "ln2wh": np.ascontiguousarray(a(inputs["ln2_w"])[:, :H].T).astype(bf16),
        "ln2wt": np.ascontiguousarray(a(inputs["ln2_w"])[:, H:].T).astype(bf16),
        "ln2b_row": a(inputs["ln2_b"]).reshape(1, H).astype(bf16),
        "ones_row": np.ones((1, BS), bf16),
        "onesH2": np.ones((H, 2), bf16),
    }
    return feeds


def make_core_feeds(inputs, core, t_steps=T):
    slc = slice(core * BS, (core + 1) * BS)
    tgt = np.asarray(inputs["targets"])[slc, :t_steps]
    hist = np.asarray(inputs["history_states"])[slc, :t_steps]
    return {
        "tgtT": np.ascontiguousarray(tgt.transpose(2, 1, 0)).astype(bf16),
        "histT": np.ascontiguousarray(hist.transpose(2, 1, 0)).astype(bf16),
    }


_nc_cache = {}


def _get_nc(t_steps=T):
    if t_steps not in _nc_cache:
        _nc_cache[t_steps] = build_nc(t_steps)
    return _nc_cache[t_steps]


def kernel(**inputs):
    nc = _get_nc(T)
    wf = make_weight_feeds(inputs)
    in_maps = [{**make_core_feeds(inputs, c), **wf} for c in range(NCORES)]
    res = run_bass_kernel_spmd(nc, in_maps, list(range(NCORES)))
    out = np.concatenate([res.results[c]["out"] for c in range(NCORES)], axis=0)
    return out.astype(np.float32)
